# revision 31
# baseline (speedup 1.0000x reference)
"""TRN2 Bass kernel for nn_LlamaAttention_45406394253674.

Llama-style attention prefill: RMSNorm -> QKV -> RoPE -> causal GQA SDPA ->
o_proj (+residual), plus kv-cache writeback.

Sharding: tensor-parallel over heads across 8 NeuronCores. Core c holds
q-heads 4c..4c+3 and kv-head c (Wq/Wk/Wv column-sharded, Wo row-sharded).
Each core computes a partial o_proj output; the host sums the 8 partials and
adds the residual (the "reduce-scatter" step done at gather time).

Matmuls run in fp16 (fast weight load + PE full rate); accumulation is fp32
in PSUM, softmax statistics in fp32. exp uses a -4 bias (softmax is
shift-invariant) so p stays well inside fp16 range.

Shapes (hardcoded per spec): L=2048, HID=4096, NH=32, NKV=8, HD=128, MAX=4096.
"""
import sys

if "/opt/trn_rl_repo" not in sys.path:
    sys.path.insert(0, "/opt/trn_rl_repo")

import numpy as np

import concourse.bacc as bacc
import concourse.tile as tile
import concourse.mybir as mybir
from concourse.bass_utils import run_bass_kernel_spmd

L = 2048
HID = 4096
NH = 32
NKV = 8
HD = 128
MAX = 4096
EPS = 1e-5
NCORES = 8
QH = NH // NCORES  # q heads per core = 4
P = 128
TG = 512  # token group (moving-dim per matmul)
NTG = L // TG  # 4
NTT = L // P  # 16 token tiles
NHC = HID // P  # 32 hid chunks
ODIM = QH * HD + 2 * HD  # 768 = 4q + k + v
NM = ODIM // P  # 6 m-tiles (0..3 q, 4 k, 5 v)
SCALE = 1.0 / np.sqrt(HD)
EXP_BIAS = -4.0
NEG = -30000.0  # additive causal mask value (fp16-representable)

F32 = mybir.dt.float32
F16 = mybir.dt.float16
BF16 = mybir.dt.bfloat16
NPW = np.float16

_CACHE = {}


def _build():
    nc = bacc.Bacc(None, target_bir_lowering=False)

    xT = nc.dram_tensor("xT", [P, NHC, L], F16, kind="ExternalInput")
    cosTs = nc.dram_tensor("cosTs", [P, L], F32, kind="ExternalInput")
    sinAs = nc.dram_tensor("sinAs", [P, L], F32, kind="ExternalInput")
    rstd_p = nc.dram_tensor("rstd_p", [P, NTT], F32, kind="ExternalInput")
    wqkvT = nc.dram_tensor("wqkvT", [P, NHC, ODIM], F16, kind="ExternalInput")
    woT = nc.dram_tensor("woT", [P, QH, HID], F16, kind="ExternalInput")
    maskT = nc.dram_tensor("maskT", [P, P], F16, kind="ExternalInput")
    ones128 = nc.dram_tensor("ones128", [P, P], BF16, kind="ExternalInput")
    ident = nc.dram_tensor("ident", [P, P], F16, kind="ExternalInput")

    out_part = nc.dram_tensor("out_part", [L, HID], F16, kind="ExternalOutput")
    kT_out = nc.dram_tensor("kT_out", [P, L], F32, kind="ExternalOutput")
    v_out = nc.dram_tensor("v_out", [L, HD], F32, kind="ExternalOutput")

    with tile.TileContext(nc) as tc:
        with (
            tc.tile_pool(name="persist", bufs=1) as persist,
            tc.tile_pool(name="xs", bufs=6) as xs,
            tc.tile_pool(name="rope", bufs=3) as rope,
            tc.tile_pool(name="pp", bufs=6) as pp,
            tc.tile_pool(name="nrm", bufs=2) as nrm,
            tc.tile_pool(name="ob", bufs=4) as ob,
        ):
            # ---- persistent tensors, split per token-group for fine deps ----
            qT = [persist.tile([P, QH, TG], F16, name=f"qT{g}") for g in range(NTG)]
            kT = [persist.tile([P, TG], F16, name=f"kT{g}") for g in range(NTG)]
            v = [persist.tile([P, HD], BF16, name=f"v{t}") for t in range(NTT)]
            attnT = [
                [persist.tile([P, TG], F16, name=f"attnT{g}_{h}") for h in range(QH)]
                for g in range(NTG)
            ]
            id_sb = persist.tile([P, P], F16)
            ones_sb = persist.tile([P, P], BF16)
            mask_sb = persist.tile([P, P], F16)
            cos_sb = [persist.tile([P, TG], F32, name=f"cos{g}") for g in range(NTG)]
            sin_sb = [persist.tile([P, TG], F32, name=f"sin{g}") for g in range(NTG)]
            rp_sb = persist.tile([P, NTT], F32)
            nc.sync.dma_start(rp_sb[:], rstd_p[:])
            wq_sb = persist.tile([P, NHC, ODIM], F16)
            wo_sb = persist.tile([P, QH, HID], F16)

            # ---------------- Phase A: QKV projection + RoPE ----------------
            with (
                tc.tile_pool(name="psA", bufs=NM + 1, space="PSUM") as psA,
                tc.tile_pool(name="psT", bufs=1, space="PSUM") as psT,
            ):
                for g in range(NTG):
                    gsl = slice(g * TG, (g + 1) * TG)
                    pm = [
                        psA.tile([P, TG], F32, tag="qkvps", name=f"qkv{m}")
                        for m in range(NM)
                    ]
                    for hc in range(NHC):
                        if g == 0:
                            nc.sync.dma_start(wq_sb[:, hc], wqkvT[:, hc])
                        xt = xs.tile([P, TG], F16, tag="xt", name="xt")
                        nc.sync.dma_start(xt[:], xT[:, hc, gsl])
                        if hc == NHC - 1:
                            # spread table loads: this g's rope tables mid-stream
                            nc.sync.dma_start(cos_sb[g][:], cosTs[:, gsl])
                            nc.sync.dma_start(sin_sb[g][:], sinAs[:, gsl])
                            if g == 0:
                                nc.sync.dma_start(id_sb[:], ident[:])
                            if g == 1:
                                nc.sync.dma_start(mask_sb[:], maskT[:])
                                nc.sync.dma_start(ones_sb[:], ones128[:])
                        for m in range(NM):
                            nc.tensor.matmul(
                                pm[m][:],
                                wq_sb[:, hc, m * P : (m + 1) * P],
                                xt[:],
                                start=(hc == 0),
                                stop=(hc == NHC - 1),
                            )
                    # RoPE on q heads and k (m = 0..4):
                    # out = qkv*cos' + swap64(qkv)*sinA'  (rstd folded in tables)
                    for m in range(QH + 1):
                        tA = rope.tile([P, TG], F32, tag="ropeA")
                        nc.vector.tensor_mul(tA[:], pm[m][:], cos_sb[g][:])
                        tB = rope.tile([P, TG], F32, tag="ropeB")
                        nc.vector.tensor_mul(
                            tB[0:64], pm[m][64:128], sin_sb[g][0:64]
                        )
                        nc.vector.tensor_mul(
                            tB[64:128], pm[m][0:64], sin_sb[g][64:128]
                        )
                        if m < QH:
                            nc.gpsimd.tensor_add(qT[g][:, m], tA[:], tB[:])
                        else:
                            k32 = rope.tile([P, TG], F32, tag="k32")
                            nc.gpsimd.tensor_add(k32[:], tA[:], tB[:])
                            nc.scalar.copy(kT[g][:], k32[:])
                            nc.sync.dma_start(kT_out[:, gsl], k32[:])
                    # v (m=5): transpose [hd, tok] -> [tok, hd], scale by rstd
                    vstage = rope.tile([P, TG], F16, tag="vstage")
                    nc.scalar.copy(vstage[:], pm[NM - 1][:])
                    for t in range(TG // P):
                        kt = g * (TG // P) + t
                        tp = psT.tile([P, P], F16, tag="vtp")
                        nc.tensor.transpose(
                            tp[:], vstage[:, t * P : (t + 1) * P], id_sb[:]
                        )
                        nc.scalar.mul(v[kt][:], tp[:], rp_sb[:, kt : kt + 1])
                        v32 = rope.tile([P, P], F32, tag="v32")
                        nc.scalar.mul(v32[:], tp[:], rp_sb[:, kt : kt + 1])
                        nc.sync.dma_start(v_out[kt * P : (kt + 1) * P], v32[:])

            # -------- Phase B+C: attention + o_proj ------
            # Group emission order interleaves the short g=0/g=1 groups (whose
            # QK->exp->PV chains are latency-bound) so they pipeline against
            # each other; o_proj(g) is emitted as soon as its attnT is ready.
            with (
                tc.tile_pool(name="psS", bufs=3, space="PSUM") as psS,
                tc.tile_pool(name="psAt", bufs=3, space="PSUM") as psAt,
                tc.tile_pool(name="psD", bufs=2, space="PSUM") as psD,
            ):
                def attn_group(h, g):
                    nkt = 4 * g + 4
                    at_ps = psAt.tile([P, TG], F32, tag="at", name="at_ps")
                    den_ps = psD.tile([P, TG], F32, tag="den", name="den_ps")
                    for kt in range(nkt):
                        dlt = kt - 4 * g  # >=0 on diagonal tiles
                        off = max(dlt, 0) * P
                        diag = dlt >= 0
                        s_ps = psS.tile([P, TG], F32, tag="s", name="s_ps")
                        nc.tensor.matmul(
                            s_ps[:, off:],
                            kT[kt // 4][:, (kt % 4) * P : (kt % 4 + 1) * P],
                            qT[g][:, h, off:],
                            start=True,
                            stop=not diag,
                        )
                        if diag:
                            nc.tensor.matmul(
                                s_ps[:, off : off + P],
                                id_sb[:],
                                mask_sb[:],
                                start=False,
                                stop=True,
                                skip_group_check=True,
                            )
                        p = pp.tile([P, TG], BF16, tag="p", name="p")
                        nc.scalar.activation(
                            p[:, off:],
                            s_ps[:, off:],
                            mybir.ActivationFunctionType.Exp,
                            scale=float(SCALE),
                        )
                        nc.tensor.matmul(
                            at_ps[:, off:],
                            v[kt][:],
                            p[:, off:],
                            start=(kt == 0),
                            stop=(kt == nkt - 1),
                            skip_group_check=True,
                        )
                        nc.tensor.matmul(
                            den_ps[:, off:],
                            ones_sb[:],
                            p[:, off:],
                            start=(kt == 0),
                            stop=(kt == nkt - 1),
                            skip_group_check=True,
                        )
                    rec = nrm.tile([P, TG], F32, tag="rec", name="rec")
                    nc.vector.reciprocal(rec[:], den_ps[:])
                    nc.vector.tensor_mul(attnT[g][h][:], at_ps[:], rec[:])

                def oproj(g, only_ts=None):
                    for ts in ([only_ts] if only_ts is not None else range(TG // P)):
                        tsl = slice(g * TG + ts * P, g * TG + (ts + 1) * P)
                        for hc in range(HID // TG):
                            o_ps = psS.tile([P, TG], F32, tag="s", name="o_ps")
                            for ac in range(QH):
                                nc.tensor.matmul(
                                    o_ps[:],
                                    attnT[g][ac][:, ts * P : (ts + 1) * P],
                                    wo_sb[:, ac, hc * TG : (hc + 1) * TG],
                                    start=(ac == 0),
                                    stop=(ac == QH - 1),
                                )
                            o_sb = ob.tile([P, TG], F16, tag="osb", name="o_sb")
                            if hc % 2 == 0:
                                nc.vector.tensor_copy(o_sb[:], o_ps[:])
                            else:
                                nc.scalar.copy(o_sb[:], o_ps[:])
                            nc.sync.dma_start(
                                out_part[tsl, hc * TG : (hc + 1) * TG], o_sb[:]
                            )

                for ac in range(QH):
                    nc.sync.dma_start(wo_sb[:, ac], woT[:, ac])
                for h in range(QH):
                    attn_group(h, 1)
                    attn_group(h, 0)
                oproj(0)
                oproj(1)
                for h in range(QH):
                    attn_group(h, 2)
                oproj(2)
                for h in range(QH):
                    attn_group(h, 3)
                oproj(3)

    nc.finalize()
    return nc


def _host_prep(x, cos, sin, ln_w, Wq, Wk, Wv, Wo):
    """Build per-core input maps. All heavy math stays on device; this is
    sharding, layout, and O(L*HID) statistics prep."""
    x = np.asarray(x, np.float32)
    cos = np.asarray(cos, np.float32)
    sin = np.asarray(sin, np.float32)
    ln_w = np.asarray(ln_w, np.float32)

    var = np.mean(x.astype(np.float64) ** 2, axis=-1) + EPS
    rstd = (1.0 / np.sqrt(var)).astype(np.float32)  # [L]

    # xT pre-tiled: [P, NHC, L]
    xT = np.ascontiguousarray(x.T.reshape(NHC, P, L).transpose(1, 0, 2)).astype(NPW)

    cosT = cos.T  # [HD, L]
    sinT = sin.T
    sinA = np.concatenate([-sinT[:64], sinT[64:]], axis=0)
    cosTs = np.ascontiguousarray(cosT * rstd[None, :])
    sinAs = np.ascontiguousarray(sinA * rstd[None, :])
    rstd_p = np.ascontiguousarray(rstd.reshape(NTT, P).T)  # [P, NTT]

    # additive causal triangle mask for the 128x128 diagonal block:
    # mask[kk, qq] = 0 if qq >= kk else NEG
    kk = np.arange(P)[:, None]
    qq = np.arange(P)[None, :]
    maskT = np.where(qq >= kk, 0.0, NEG).astype(NPW)

    import ml_dtypes
    ones128 = np.ones((P, P), ml_dtypes.bfloat16)
    ident = np.eye(P, dtype=NPW)

    Wqf = np.asarray(Wq, np.float32) * ln_w[None, :]
    Wkf = np.asarray(Wk, np.float32) * ln_w[None, :]
    Wvf = np.asarray(Wv, np.float32) * ln_w[None, :]
    Wo = np.asarray(Wo, np.float32)

    in_maps = []
    for c in range(NCORES):
        qsl = slice(c * QH * HD, (c + 1) * QH * HD)
        ksl = slice(c * HD, (c + 1) * HD)
        Wcat = np.concatenate([Wqf[qsl], Wkf[ksl], Wvf[ksl]], axis=0)  # [768, HID]
        wqkvT = np.ascontiguousarray(
            Wcat.T.reshape(NHC, P, ODIM).transpose(1, 0, 2)
        ).astype(NPW)
        WoT = Wo[:, qsl].T  # [512, HID]
        woT = np.ascontiguousarray(
            WoT.reshape(QH, P, HID).transpose(1, 0, 2)
        ).astype(NPW)
        in_maps.append(
            {
                "xT": xT,
                "cosTs": cosTs,
                "sinAs": sinAs,
                "rstd_p": rstd_p,
                "wqkvT": wqkvT,
                "woT": woT,
                "maskT": maskT,
                "ones128": ones128,
                "ident": ident,
            }
        )
    return in_maps


def kernel(
    x,
    cos,
    sin,
    position_ids,
    seq_len,
    k_cache,
    v_cache,
    ln_w,
    Wq,
    Wk,
    Wv,
    Wo,
    _trace=False,
):
    if "nc" not in _CACHE:
        _CACHE["nc"] = _build()
    nc = _CACHE["nc"]

    in_maps = _host_prep(x, cos, sin, ln_w, Wq, Wk, Wv, Wo)
    res = run_bass_kernel_spmd(
        nc, in_maps, core_ids=list(range(NCORES)), trace=_trace
    )
    _CACHE["last_result"] = res

    x = np.asarray(x, np.float32)
    out = x.copy()
    for c in range(NCORES):
        out += res.results[c]["out_part"].astype(np.float32)

    pos = np.asarray(position_ids).astype(np.int64)
    k_cache_out = np.array(np.asarray(k_cache, np.float32), copy=True)
    v_cache_out = np.array(np.asarray(v_cache, np.float32), copy=True)
    for c in range(NCORES):
        k_cache_out[pos, c, :] = res.results[c]["kT_out"].T
        v_cache_out[pos, c, :] = res.results[c]["v_out"]
    return out, k_cache_out, v_cache_out


# revision 32
# speedup vs baseline: 1.0282x; 1.0282x over previous
"""TRN2 Bass kernel for nn_LlamaAttention_45406394253674.

Llama-style attention prefill: RMSNorm -> QKV -> RoPE -> causal GQA SDPA ->
o_proj (+residual), plus kv-cache writeback.

Sharding: tensor-parallel over heads across 8 NeuronCores. Core c holds
q-heads 4c..4c+3 and kv-head c (Wq/Wk/Wv column-sharded, Wo row-sharded).
Each core computes a partial o_proj output; the host sums the 8 partials and
adds the residual (the "reduce-scatter" step done at gather time).

Matmuls run in fp16 (fast weight load + PE full rate); accumulation is fp32
in PSUM, softmax statistics in fp32. exp uses a -4 bias (softmax is
shift-invariant) so p stays well inside fp16 range.

Shapes (hardcoded per spec): L=2048, HID=4096, NH=32, NKV=8, HD=128, MAX=4096.
"""
import sys

if "/opt/trn_rl_repo" not in sys.path:
    sys.path.insert(0, "/opt/trn_rl_repo")

import numpy as np

import concourse.bacc as bacc
import concourse.tile as tile
import concourse.mybir as mybir
from concourse.bass_utils import run_bass_kernel_spmd

L = 2048
HID = 4096
NH = 32
NKV = 8
HD = 128
MAX = 4096
EPS = 1e-5
NCORES = 8
QH = NH // NCORES  # q heads per core = 4
P = 128
TG = 512  # token group (moving-dim per matmul)
NTG = L // TG  # 4
NTT = L // P  # 16 token tiles
NHC = HID // P  # 32 hid chunks
ODIM = QH * HD + 2 * HD  # 768 = 4q + k + v
NM = ODIM // P  # 6 m-tiles (0..3 q, 4 k, 5 v)
SCALE = 1.0 / np.sqrt(HD)
EXP_BIAS = -4.0
NEG = -30000.0  # additive causal mask value (fp16-representable)

F32 = mybir.dt.float32
F16 = mybir.dt.float16
BF16 = mybir.dt.bfloat16
NPW = np.float16

_CACHE = {}


def _build():
    nc = bacc.Bacc(None, target_bir_lowering=False)

    xT = nc.dram_tensor("xT", [P, NHC, L], F16, kind="ExternalInput")
    cosTs = nc.dram_tensor("cosTs", [P, L], F32, kind="ExternalInput")
    sinAs = nc.dram_tensor("sinAs", [P, L], F32, kind="ExternalInput")
    rstd_p = nc.dram_tensor("rstd_p", [P, NTT], F32, kind="ExternalInput")
    wqkvT = nc.dram_tensor("wqkvT", [P, NHC, ODIM], F16, kind="ExternalInput")
    woT = nc.dram_tensor("woT", [P, QH, HID], F16, kind="ExternalInput")
    maskT = nc.dram_tensor("maskT", [P, P], F16, kind="ExternalInput")
    ones128 = nc.dram_tensor("ones128", [P, P], BF16, kind="ExternalInput")
    ident = nc.dram_tensor("ident", [P, P], F16, kind="ExternalInput")

    out_part = nc.dram_tensor("out_part", [L, HID], F16, kind="ExternalOutput")
    kT_out = nc.dram_tensor("kT_out", [P, L], F32, kind="ExternalOutput")
    v_out = nc.dram_tensor("v_out", [L, HD], F32, kind="ExternalOutput")

    with tile.TileContext(nc) as tc:
        with (
            tc.tile_pool(name="persist", bufs=1) as persist,
            tc.tile_pool(name="xs", bufs=6) as xs,
            tc.tile_pool(name="rope", bufs=3) as rope,
            tc.tile_pool(name="pp", bufs=6) as pp,
            tc.tile_pool(name="nrm", bufs=4) as nrm,
            tc.tile_pool(name="ob", bufs=4) as ob,
        ):
            # ---- persistent tensors, split per token-group for fine deps ----
            qT = [persist.tile([P, QH, TG], F16, name=f"qT{g}") for g in range(NTG)]
            kT = [persist.tile([P, TG], F16, name=f"kT{g}") for g in range(NTG)]
            v = [persist.tile([P, HD], BF16, name=f"v{t}") for t in range(NTT)]
            attnT = [
                [persist.tile([P, TG], F16, name=f"attnT{g}_{h}") for h in range(QH)]
                for g in range(NTG)
            ]
            id_sb = persist.tile([P, P], F16)
            ones_sb = persist.tile([P, P], BF16)
            mask_sb = persist.tile([P, P], F16)
            cos_sb = [persist.tile([P, TG], F32, name=f"cos{g}") for g in range(NTG)]
            sin_sb = [persist.tile([P, TG], F32, name=f"sin{g}") for g in range(NTG)]
            rp_sb = persist.tile([P, NTT], F32)
            nc.sync.dma_start(rp_sb[:], rstd_p[:])
            wq_sb = persist.tile([P, NHC, ODIM], F16)
            wo_sb = persist.tile([P, QH, HID], F16)

            # ---------------- Phase A: QKV projection + RoPE ----------------
            with (
                tc.tile_pool(name="psA", bufs=NM + 1, space="PSUM") as psA,
                tc.tile_pool(name="psT", bufs=1, space="PSUM") as psT,
            ):
                for g in range(NTG):
                    gsl = slice(g * TG, (g + 1) * TG)
                    pm = [
                        psA.tile([P, TG], F32, tag="qkvps", name=f"qkv{m}")
                        for m in range(NM)
                    ]
                    for hc in range(NHC):
                        if g == 0:
                            nc.sync.dma_start(wq_sb[:, hc], wqkvT[:, hc])
                        xt = xs.tile([P, TG], F16, tag="xt", name="xt")
                        nc.sync.dma_start(xt[:], xT[:, hc, gsl])
                        if hc == NHC - 1:
                            # spread table loads: this g's rope tables mid-stream
                            nc.sync.dma_start(cos_sb[g][:], cosTs[:, gsl])
                            nc.sync.dma_start(sin_sb[g][:], sinAs[:, gsl])
                            if g == 0:
                                nc.sync.dma_start(id_sb[:], ident[:])
                            if g == 1:
                                nc.sync.dma_start(mask_sb[:], maskT[:])
                                nc.sync.dma_start(ones_sb[:], ones128[:])
                        for m in range(NM):
                            nc.tensor.matmul(
                                pm[m][:],
                                wq_sb[:, hc, m * P : (m + 1) * P],
                                xt[:],
                                start=(hc == 0),
                                stop=(hc == NHC - 1),
                            )
                    # RoPE on q heads and k (m = 0..4):
                    # out = qkv*cos' + swap64(qkv)*sinA'  (rstd folded in tables)
                    for m in range(QH + 1):
                        tA = rope.tile([P, TG], F32, tag="ropeA")
                        nc.vector.tensor_mul(tA[:], pm[m][:], cos_sb[g][:])
                        tB = rope.tile([P, TG], F32, tag="ropeB")
                        nc.vector.tensor_mul(
                            tB[0:64], pm[m][64:128], sin_sb[g][0:64]
                        )
                        nc.vector.tensor_mul(
                            tB[64:128], pm[m][0:64], sin_sb[g][64:128]
                        )
                        if m < QH:
                            nc.gpsimd.tensor_add(qT[g][:, m], tA[:], tB[:])
                        else:
                            k32 = rope.tile([P, TG], F32, tag="k32")
                            nc.gpsimd.tensor_add(k32[:], tA[:], tB[:])
                            nc.scalar.copy(kT[g][:], k32[:])
                            nc.sync.dma_start(kT_out[:, gsl], k32[:])
                    # v (m=5): transpose [hd, tok] -> [tok, hd], scale by rstd
                    vstage = rope.tile([P, TG], F16, tag="vstage")
                    nc.scalar.copy(vstage[:], pm[NM - 1][:])
                    for t in range(TG // P):
                        kt = g * (TG // P) + t
                        tp = psT.tile([P, P], F16, tag="vtp")
                        nc.tensor.transpose(
                            tp[:], vstage[:, t * P : (t + 1) * P], id_sb[:]
                        )
                        nc.scalar.mul(v[kt][:], tp[:], rp_sb[:, kt : kt + 1])
                        v32 = rope.tile([P, P], F32, tag="v32")
                        nc.scalar.mul(v32[:], tp[:], rp_sb[:, kt : kt + 1])
                        nc.sync.dma_start(v_out[kt * P : (kt + 1) * P], v32[:])

            # -------- Phase B+C: attention + o_proj ------
            # Group emission order interleaves the short g=0/g=1 groups (whose
            # QK->exp->PV chains are latency-bound) so they pipeline against
            # each other; o_proj(g) is emitted as soon as its attnT is ready.
            with (
                tc.tile_pool(name="psS", bufs=3, space="PSUM") as psS,
                tc.tile_pool(name="psAt", bufs=3, space="PSUM") as psAt,
                tc.tile_pool(name="psD", bufs=2, space="PSUM") as psD,
            ):
                def attn_group(h, g):
                    nkt = 4 * g + 4
                    at_ps = psAt.tile([P, TG], F32, tag="at", name="at_ps")
                    den_ps = psD.tile([P, TG], F32, tag="den", name="den_ps")
                    for kt in range(nkt):
                        dlt = kt - 4 * g  # >=0 on diagonal tiles
                        off = max(dlt, 0) * P
                        diag = dlt >= 0
                        s_ps = psS.tile([P, TG], F32, tag="s", name="s_ps")
                        nc.tensor.matmul(
                            s_ps[:, off:],
                            kT[kt // 4][:, (kt % 4) * P : (kt % 4 + 1) * P],
                            qT[g][:, h, off:],
                            start=True,
                            stop=not diag,
                        )
                        if diag:
                            nc.tensor.matmul(
                                s_ps[:, off : off + P],
                                id_sb[:],
                                mask_sb[:],
                                start=False,
                                stop=True,
                                skip_group_check=True,
                            )
                        p = pp.tile([P, TG], BF16, tag="p", name="p")
                        nc.scalar.activation(
                            p[:, off:],
                            s_ps[:, off:],
                            mybir.ActivationFunctionType.Exp,
                            scale=float(SCALE),
                        )
                        nc.tensor.matmul(
                            at_ps[:, off:],
                            v[kt][:],
                            p[:, off:],
                            start=(kt == 0),
                            stop=(kt == nkt - 1),
                            skip_group_check=True,
                        )
                        nc.tensor.matmul(
                            den_ps[:, off:],
                            ones_sb[:],
                            p[:, off:],
                            start=(kt == 0),
                            stop=(kt == nkt - 1),
                            skip_group_check=True,
                        )
                    rec = nrm.tile([P, TG], F32, tag="rec", name="rec")
                    nc.vector.reciprocal(rec[:], den_ps[:])
                    nc.vector.tensor_mul(attnT[g][h][:], at_ps[:], rec[:])

                def oproj(g, only_ts=None):
                    for ts in ([only_ts] if only_ts is not None else range(TG // P)):
                        tsl = slice(g * TG + ts * P, g * TG + (ts + 1) * P)
                        for hc in range(HID // TG):
                            o_ps = psS.tile([P, TG], F32, tag="s", name="o_ps")
                            for ac in range(QH):
                                nc.tensor.matmul(
                                    o_ps[:],
                                    attnT[g][ac][:, ts * P : (ts + 1) * P],
                                    wo_sb[:, ac, hc * TG : (hc + 1) * TG],
                                    start=(ac == 0),
                                    stop=(ac == QH - 1),
                                )
                            o_sb = ob.tile([P, TG], F16, tag="osb", name="o_sb")
                            if hc % 2 == 0:
                                nc.vector.tensor_copy(o_sb[:], o_ps[:])
                            else:
                                nc.scalar.copy(o_sb[:], o_ps[:])
                            nc.sync.dma_start(
                                out_part[tsl, hc * TG : (hc + 1) * TG], o_sb[:]
                            )

                for ac in range(QH):
                    nc.sync.dma_start(wo_sb[:, ac], woT[:, ac])
                for h in range(QH):
                    attn_group(h, 0)
                    attn_group(h, 1)
                oproj(0)
                oproj(1)
                for h in range(QH):
                    attn_group(h, 2)
                attn_group(0, 3)
                oproj(2)
                for h in range(1, QH):
                    attn_group(h, 3)
                oproj(3)

    nc.finalize()
    return nc


def _host_prep(x, cos, sin, ln_w, Wq, Wk, Wv, Wo):
    """Build per-core input maps. All heavy math stays on device; this is
    sharding, layout, and O(L*HID) statistics prep."""
    x = np.asarray(x, np.float32)
    cos = np.asarray(cos, np.float32)
    sin = np.asarray(sin, np.float32)
    ln_w = np.asarray(ln_w, np.float32)

    var = np.mean(x.astype(np.float64) ** 2, axis=-1) + EPS
    rstd = (1.0 / np.sqrt(var)).astype(np.float32)  # [L]

    # xT pre-tiled: [P, NHC, L]
    xT = np.ascontiguousarray(x.T.reshape(NHC, P, L).transpose(1, 0, 2)).astype(NPW)

    cosT = cos.T  # [HD, L]
    sinT = sin.T
    sinA = np.concatenate([-sinT[:64], sinT[64:]], axis=0)
    cosTs = np.ascontiguousarray(cosT * rstd[None, :])
    sinAs = np.ascontiguousarray(sinA * rstd[None, :])
    rstd_p = np.ascontiguousarray(rstd.reshape(NTT, P).T)  # [P, NTT]

    # additive causal triangle mask for the 128x128 diagonal block:
    # mask[kk, qq] = 0 if qq >= kk else NEG
    kk = np.arange(P)[:, None]
    qq = np.arange(P)[None, :]
    maskT = np.where(qq >= kk, 0.0, NEG).astype(NPW)

    import ml_dtypes
    ones128 = np.ones((P, P), ml_dtypes.bfloat16)
    ident = np.eye(P, dtype=NPW)

    Wqf = np.asarray(Wq, np.float32) * ln_w[None, :]
    Wkf = np.asarray(Wk, np.float32) * ln_w[None, :]
    Wvf = np.asarray(Wv, np.float32) * ln_w[None, :]
    Wo = np.asarray(Wo, np.float32)

    in_maps = []
    for c in range(NCORES):
        qsl = slice(c * QH * HD, (c + 1) * QH * HD)
        ksl = slice(c * HD, (c + 1) * HD)
        Wcat = np.concatenate([Wqf[qsl], Wkf[ksl], Wvf[ksl]], axis=0)  # [768, HID]
        wqkvT = np.ascontiguousarray(
            Wcat.T.reshape(NHC, P, ODIM).transpose(1, 0, 2)
        ).astype(NPW)
        WoT = Wo[:, qsl].T  # [512, HID]
        woT = np.ascontiguousarray(
            WoT.reshape(QH, P, HID).transpose(1, 0, 2)
        ).astype(NPW)
        in_maps.append(
            {
                "xT": xT,
                "cosTs": cosTs,
                "sinAs": sinAs,
                "rstd_p": rstd_p,
                "wqkvT": wqkvT,
                "woT": woT,
                "maskT": maskT,
                "ones128": ones128,
                "ident": ident,
            }
        )
    return in_maps


def kernel(
    x,
    cos,
    sin,
    position_ids,
    seq_len,
    k_cache,
    v_cache,
    ln_w,
    Wq,
    Wk,
    Wv,
    Wo,
    _trace=False,
):
    if "nc" not in _CACHE:
        _CACHE["nc"] = _build()
    nc = _CACHE["nc"]

    in_maps = _host_prep(x, cos, sin, ln_w, Wq, Wk, Wv, Wo)
    res = run_bass_kernel_spmd(
        nc, in_maps, core_ids=list(range(NCORES)), trace=_trace
    )
    _CACHE["last_result"] = res

    x = np.asarray(x, np.float32)
    out = x.copy()
    for c in range(NCORES):
        out += res.results[c]["out_part"].astype(np.float32)

    pos = np.asarray(position_ids).astype(np.int64)
    k_cache_out = np.array(np.asarray(k_cache, np.float32), copy=True)
    v_cache_out = np.array(np.asarray(v_cache, np.float32), copy=True)
    for c in range(NCORES):
        k_cache_out[pos, c, :] = res.results[c]["kT_out"].T
        v_cache_out[pos, c, :] = res.results[c]["v_out"]
    return out, k_cache_out, v_cache_out


# revision 33
# speedup vs baseline: 1.0324x; 1.0041x over previous
"""TRN2 Bass kernel for nn_LlamaAttention_45406394253674.

Llama-style attention prefill: RMSNorm -> QKV -> RoPE -> causal GQA SDPA ->
o_proj (+residual), plus kv-cache writeback.

Sharding: tensor-parallel over heads across 8 NeuronCores. Core c holds
q-heads 4c..4c+3 and kv-head c (Wq/Wk/Wv column-sharded, Wo row-sharded).
Each core computes a partial o_proj output; the host sums the 8 partials and
adds the residual (the "reduce-scatter" step done at gather time).

Matmuls run in fp16 (fast weight load + PE full rate); accumulation is fp32
in PSUM, softmax statistics in fp32. exp uses a -4 bias (softmax is
shift-invariant) so p stays well inside fp16 range.

Shapes (hardcoded per spec): L=2048, HID=4096, NH=32, NKV=8, HD=128, MAX=4096.
"""
import sys

if "/opt/trn_rl_repo" not in sys.path:
    sys.path.insert(0, "/opt/trn_rl_repo")

import numpy as np

import concourse.bacc as bacc
import concourse.tile as tile
import concourse.mybir as mybir
from concourse.bass_utils import run_bass_kernel_spmd

L = 2048
HID = 4096
NH = 32
NKV = 8
HD = 128
MAX = 4096
EPS = 1e-5
NCORES = 8
QH = NH // NCORES  # q heads per core = 4
P = 128
TG = 512  # token group (moving-dim per matmul)
NTG = L // TG  # 4
NTT = L // P  # 16 token tiles
NHC = HID // P  # 32 hid chunks
ODIM = QH * HD + 2 * HD  # 768 = 4q + k + v
NM = ODIM // P  # 6 m-tiles (0..3 q, 4 k, 5 v)
SCALE = 1.0 / np.sqrt(HD)
EXP_BIAS = -4.0
NEG = -30000.0  # additive causal mask value (fp16-representable)

F32 = mybir.dt.float32
F16 = mybir.dt.float16
BF16 = mybir.dt.bfloat16
NPW = np.float16

_CACHE = {}


def _build():
    nc = bacc.Bacc(None, target_bir_lowering=False)

    xT = nc.dram_tensor("xT", [P, NHC, L], F16, kind="ExternalInput")
    cosTs = nc.dram_tensor("cosTs", [P, L], F32, kind="ExternalInput")
    sinAs = nc.dram_tensor("sinAs", [P, L], F32, kind="ExternalInput")
    rstd_p = nc.dram_tensor("rstd_p", [P, NTT], F32, kind="ExternalInput")
    wqkvT = nc.dram_tensor("wqkvT", [P, NHC, ODIM], F16, kind="ExternalInput")
    woT = nc.dram_tensor("woT", [P, QH, HID], F16, kind="ExternalInput")
    maskT = nc.dram_tensor("maskT", [P, P], F16, kind="ExternalInput")
    ones128 = nc.dram_tensor("ones128", [P, P], BF16, kind="ExternalInput")
    ident = nc.dram_tensor("ident", [P, P], F16, kind="ExternalInput")

    out_part = nc.dram_tensor("out_part", [L, HID], F16, kind="ExternalOutput")
    kT_out = nc.dram_tensor("kT_out", [P, L], F32, kind="ExternalOutput")
    v_out = nc.dram_tensor("v_out", [L, HD], F32, kind="ExternalOutput")

    with tile.TileContext(nc) as tc:
        with (
            tc.tile_pool(name="persist", bufs=1) as persist,
            tc.tile_pool(name="xs", bufs=6) as xs,
            tc.tile_pool(name="rope", bufs=3) as rope,
            tc.tile_pool(name="pp", bufs=6) as pp,
            tc.tile_pool(name="nrm", bufs=4) as nrm,
            tc.tile_pool(name="ob", bufs=4) as ob,
        ):
            # ---- persistent tensors, split per token-group for fine deps ----
            qT = [persist.tile([P, QH, TG], F16, name=f"qT{g}") for g in range(NTG)]
            kT = [persist.tile([P, TG], F16, name=f"kT{g}") for g in range(NTG)]
            v = [persist.tile([P, HD], BF16, name=f"v{t}") for t in range(NTT)]
            attnT = [
                [persist.tile([P, TG], F16, name=f"attnT{g}_{h}") for h in range(QH)]
                for g in range(NTG)
            ]
            id_sb = persist.tile([P, P], F16)
            ones_sb = persist.tile([P, P], BF16)
            mask_sb = persist.tile([P, P], F16)
            cos_sb = [persist.tile([P, TG], F32, name=f"cos{g}") for g in range(NTG)]
            sin_sb = [persist.tile([P, TG], F32, name=f"sin{g}") for g in range(NTG)]
            rp_sb = persist.tile([P, NTT], F32)
            nc.sync.dma_start(rp_sb[:], rstd_p[:])
            wq_sb = persist.tile([P, NHC, ODIM], F16)
            wo_sb = persist.tile([P, QH, HID], F16)

            # ---------------- Phase A: QKV projection + RoPE ----------------
            with (
                tc.tile_pool(name="psA", bufs=NM + 2, space="PSUM") as psA,
            ):
                for g in range(NTG):
                    gsl = slice(g * TG, (g + 1) * TG)
                    pm = [
                        psA.tile([P, TG], F32, tag="qkvps", name=f"qkv{m}")
                        for m in range(NM)
                    ]
                    for hc in range(NHC):
                        if g == 0:
                            nc.sync.dma_start(wq_sb[:, hc], wqkvT[:, hc])
                        xt = xs.tile([P, TG], F16, tag="xt", name="xt")
                        nc.sync.dma_start(xt[:], xT[:, hc, gsl])
                        if hc == NHC - 1:
                            # spread table loads: this g's rope tables mid-stream
                            nc.sync.dma_start(cos_sb[g][:], cosTs[:, gsl])
                            nc.sync.dma_start(sin_sb[g][:], sinAs[:, gsl])
                            if g == 0:
                                nc.sync.dma_start(id_sb[:], ident[:])
                            if g == 1:
                                nc.sync.dma_start(mask_sb[:], maskT[:])
                                nc.sync.dma_start(ones_sb[:], ones128[:])
                        for m in range(NM):
                            nc.tensor.matmul(
                                pm[m][:],
                                wq_sb[:, hc, m * P : (m + 1) * P],
                                xt[:],
                                start=(hc == 0),
                                stop=(hc == NHC - 1),
                            )
                    # RoPE on q heads and k (m = 0..4):
                    # out = qkv*cos' + swap64(qkv)*sinA'  (rstd folded in tables)
                    for m in range(QH + 1):
                        tA = rope.tile([P, TG], F32, tag="ropeA")
                        nc.vector.tensor_mul(tA[:], pm[m][:], cos_sb[g][:])
                        tB = rope.tile([P, TG], F32, tag="ropeB")
                        nc.vector.tensor_mul(
                            tB[0:64], pm[m][64:128], sin_sb[g][0:64]
                        )
                        nc.vector.tensor_mul(
                            tB[64:128], pm[m][0:64], sin_sb[g][64:128]
                        )
                        if m < QH:
                            nc.gpsimd.tensor_add(qT[g][:, m], tA[:], tB[:])
                        else:
                            k32 = rope.tile([P, TG], F32, tag="k32")
                            nc.gpsimd.tensor_add(k32[:], tA[:], tB[:])
                            nc.scalar.copy(kT[g][:], k32[:])
                            nc.sync.dma_start(kT_out[:, gsl], k32[:])
                    # v (m=5): transpose [hd, tok] -> [tok, hd], scale by rstd
                    vstage = rope.tile([P, TG], F16, tag="vstage")
                    nc.scalar.copy(vstage[:], pm[NM - 1][:])
                    for t in range(TG // P):
                        kt = g * (TG // P) + t
                        tp = psA.tile([P, P], F16, tag="qkvps", name="vtp")
                        nc.tensor.transpose(
                            tp[:], vstage[:, t * P : (t + 1) * P], id_sb[:]
                        )
                        nc.scalar.mul(v[kt][:], tp[:], rp_sb[:, kt : kt + 1])
                        v32 = rope.tile([P, P], F32, tag="v32")
                        nc.scalar.mul(v32[:], tp[:], rp_sb[:, kt : kt + 1])
                        nc.sync.dma_start(v_out[kt * P : (kt + 1) * P], v32[:])

            # -------- Phase B+C: attention + o_proj ------
            # Group emission order interleaves the short g=0/g=1 groups (whose
            # QK->exp->PV chains are latency-bound) so they pipeline against
            # each other; o_proj(g) is emitted as soon as its attnT is ready.
            with (
                tc.tile_pool(name="psS", bufs=3, space="PSUM") as psS,
                tc.tile_pool(name="psAt", bufs=3, space="PSUM") as psAt,
                tc.tile_pool(name="psD", bufs=2, space="PSUM") as psD,
            ):
                def attn_group(h, g):
                    nkt = 4 * g + 4
                    at_ps = psAt.tile([P, TG], F32, tag="at", name="at_ps")
                    den_ps = psD.tile([P, TG], F32, tag="den", name="den_ps")
                    for kt in range(nkt):
                        dlt = kt - 4 * g  # >=0 on diagonal tiles
                        off = max(dlt, 0) * P
                        diag = dlt >= 0
                        s_ps = psS.tile([P, TG], F32, tag="s", name="s_ps")
                        nc.tensor.matmul(
                            s_ps[:, off:],
                            kT[kt // 4][:, (kt % 4) * P : (kt % 4 + 1) * P],
                            qT[g][:, h, off:],
                            start=True,
                            stop=not diag,
                        )
                        if diag:
                            nc.tensor.matmul(
                                s_ps[:, off : off + P],
                                id_sb[:],
                                mask_sb[:],
                                start=False,
                                stop=True,
                                skip_group_check=True,
                            )
                        p = pp.tile([P, TG], BF16, tag="p", name="p")
                        nc.scalar.activation(
                            p[:, off:],
                            s_ps[:, off:],
                            mybir.ActivationFunctionType.Exp,
                            scale=float(SCALE),
                        )
                        nc.tensor.matmul(
                            at_ps[:, off:],
                            v[kt][:],
                            p[:, off:],
                            start=(kt == 0),
                            stop=(kt == nkt - 1),
                            skip_group_check=True,
                        )
                        nc.tensor.matmul(
                            den_ps[:, off:],
                            ones_sb[:],
                            p[:, off:],
                            start=(kt == 0),
                            stop=(kt == nkt - 1),
                            skip_group_check=True,
                        )
                    rec = nrm.tile([P, TG], F32, tag="rec", name="rec")
                    nc.vector.reciprocal(rec[:], den_ps[:])
                    nc.vector.tensor_mul(attnT[g][h][:], at_ps[:], rec[:])

                def oproj(g, only_ts=None):
                    for ts in ([only_ts] if only_ts is not None else range(TG // P)):
                        tsl = slice(g * TG + ts * P, g * TG + (ts + 1) * P)
                        for hc in range(HID // TG):
                            o_ps = psS.tile([P, TG], F32, tag="s", name="o_ps")
                            for ac in range(QH):
                                nc.tensor.matmul(
                                    o_ps[:],
                                    attnT[g][ac][:, ts * P : (ts + 1) * P],
                                    wo_sb[:, ac, hc * TG : (hc + 1) * TG],
                                    start=(ac == 0),
                                    stop=(ac == QH - 1),
                                )
                            o_sb = ob.tile([P, TG], F16, tag="osb", name="o_sb")
                            if hc % 2 == 0:
                                nc.vector.tensor_copy(o_sb[:], o_ps[:])
                            else:
                                nc.scalar.copy(o_sb[:], o_ps[:])
                            nc.sync.dma_start(
                                out_part[tsl, hc * TG : (hc + 1) * TG], o_sb[:]
                            )

                for ac in range(QH):
                    nc.sync.dma_start(wo_sb[:, ac], woT[:, ac])
                for h in range(QH):
                    attn_group(h, 0)
                    attn_group(h, 1)
                oproj(0)
                oproj(1)
                for h in range(QH):
                    attn_group(h, 2)
                attn_group(0, 3)
                oproj(2)
                for h in range(1, QH):
                    attn_group(h, 3)
                oproj(3)

    nc.finalize()
    return nc


def _host_prep(x, cos, sin, ln_w, Wq, Wk, Wv, Wo):
    """Build per-core input maps. All heavy math stays on device; this is
    sharding, layout, and O(L*HID) statistics prep."""
    x = np.asarray(x, np.float32)
    cos = np.asarray(cos, np.float32)
    sin = np.asarray(sin, np.float32)
    ln_w = np.asarray(ln_w, np.float32)

    var = np.mean(x.astype(np.float64) ** 2, axis=-1) + EPS
    rstd = (1.0 / np.sqrt(var)).astype(np.float32)  # [L]

    # xT pre-tiled: [P, NHC, L]
    xT = np.ascontiguousarray(x.T.reshape(NHC, P, L).transpose(1, 0, 2)).astype(NPW)

    cosT = cos.T  # [HD, L]
    sinT = sin.T
    sinA = np.concatenate([-sinT[:64], sinT[64:]], axis=0)
    cosTs = np.ascontiguousarray(cosT * rstd[None, :])
    sinAs = np.ascontiguousarray(sinA * rstd[None, :])
    rstd_p = np.ascontiguousarray(rstd.reshape(NTT, P).T)  # [P, NTT]

    # additive causal triangle mask for the 128x128 diagonal block:
    # mask[kk, qq] = 0 if qq >= kk else NEG
    kk = np.arange(P)[:, None]
    qq = np.arange(P)[None, :]
    maskT = np.where(qq >= kk, 0.0, NEG).astype(NPW)

    import ml_dtypes
    ones128 = np.ones((P, P), ml_dtypes.bfloat16)
    ident = np.eye(P, dtype=NPW)

    Wqf = np.asarray(Wq, np.float32) * ln_w[None, :]
    Wkf = np.asarray(Wk, np.float32) * ln_w[None, :]
    Wvf = np.asarray(Wv, np.float32) * ln_w[None, :]
    Wo = np.asarray(Wo, np.float32)

    in_maps = []
    for c in range(NCORES):
        qsl = slice(c * QH * HD, (c + 1) * QH * HD)
        ksl = slice(c * HD, (c + 1) * HD)
        Wcat = np.concatenate([Wqf[qsl], Wkf[ksl], Wvf[ksl]], axis=0)  # [768, HID]
        wqkvT = np.ascontiguousarray(
            Wcat.T.reshape(NHC, P, ODIM).transpose(1, 0, 2)
        ).astype(NPW)
        WoT = Wo[:, qsl].T  # [512, HID]
        woT = np.ascontiguousarray(
            WoT.reshape(QH, P, HID).transpose(1, 0, 2)
        ).astype(NPW)
        in_maps.append(
            {
                "xT": xT,
                "cosTs": cosTs,
                "sinAs": sinAs,
                "rstd_p": rstd_p,
                "wqkvT": wqkvT,
                "woT": woT,
                "maskT": maskT,
                "ones128": ones128,
                "ident": ident,
            }
        )
    return in_maps


def kernel(
    x,
    cos,
    sin,
    position_ids,
    seq_len,
    k_cache,
    v_cache,
    ln_w,
    Wq,
    Wk,
    Wv,
    Wo,
    _trace=False,
):
    if "nc" not in _CACHE:
        _CACHE["nc"] = _build()
    nc = _CACHE["nc"]

    in_maps = _host_prep(x, cos, sin, ln_w, Wq, Wk, Wv, Wo)
    res = run_bass_kernel_spmd(
        nc, in_maps, core_ids=list(range(NCORES)), trace=_trace
    )
    _CACHE["last_result"] = res

    x = np.asarray(x, np.float32)
    out = x.copy()
    for c in range(NCORES):
        out += res.results[c]["out_part"].astype(np.float32)

    pos = np.asarray(position_ids).astype(np.int64)
    k_cache_out = np.array(np.asarray(k_cache, np.float32), copy=True)
    v_cache_out = np.array(np.asarray(v_cache, np.float32), copy=True)
    for c in range(NCORES):
        k_cache_out[pos, c, :] = res.results[c]["kT_out"].T
        v_cache_out[pos, c, :] = res.results[c]["v_out"]
    return out, k_cache_out, v_cache_out
